# revision 4
# baseline (speedup 1.0000x reference)
"""ArcMarginProduct (ArcFace) forward on 8 TRN2 NeuronCores.

out[b, c] = s * cos(theta_bc)         except at c == label[b] where
out[b, c] = s * phi(cos(theta_bc))    (margin epilogue)

Strategy (classification-parallel / Partial-FC), [batch, class] orientation:
  - pad C 84281 -> 84992 = 8 * 10624 class columns, shard across 8 cores
  - host ships each core its weight shard TRANSPOSED: wt [D=512, CS=10624]
    f32 (pure layout change), plus wlab = weight[label] [B, D] for the
    margin path; labels never gathered on device
  - device: f32r matmuls (1 cycle/row, ~tf32 precision, no bf16 casts):
      po[b, c]  = sum_d xnT[d, b]^T wt[d, c]     (x stationary, w moving)
      nrm[:, c] = sum_d wt[d, c]^2               (all-ones stationary ->
                                                  row-replicated norms, so the
                                                  epilogue needs no broadcast)
      out[b, c] = po * (s / sqrt(nrm))           (one DVE op per tile, bf16)
  - margin: cos/phi for all 512 labels from wlab dots; 512-element indirect
    DMA scatter overwrites out[b, label[b]] at the end
  - host concatenates shards along class axis, drops padding, casts f32

Per-core engine budget (cost-model):  DMA ~98us (bound: 21.75MB weight read
f32 + 10.9MB bf16 out), PE ~90us, DVE ~60us, ACT ~50us.
"""

import math

import numpy as np

B = 512
D = 512
C = 84281
NCORES = 8
CS = 10624          # padded classes per core (83 * 128)
REAL = [10536] * 7 + [C - 10536 * 7]   # real class cols per core
BASE = [10536 * i for i in range(NCORES)]
PAD_COL = CS - 1    # always-padding column, scatter dump for out-of-range
CHUNKS = [512] * 20 + [384]            # class chunks per core (sum = CS)

S_SCALE = 32.0
MARGIN = 0.5
COS_M = math.cos(MARGIN)
SIN_M = math.sin(MARGIN)
TH = math.cos(math.pi - MARGIN)
MM = math.sin(math.pi - MARGIN) * MARGIN

_CACHE = {}


def _build_nc(with_scatter=True):
    import concourse.tile as tile
    from concourse import bacc, mybir
    from concourse.bass import IndirectOffsetOnAxis
    from contextlib import ExitStack

    f32 = mybir.dt.float32
    f32r = mybir.dt.float32r
    bf16 = mybir.dt.bfloat16
    i32 = mybir.dt.int32
    Act = mybir.ActivationFunctionType
    Alu = mybir.AluOpType

    nc = bacc.Bacc("TRN2", target_bir_lowering=False, debug=False, num_devices=NCORES)
    x_ext = nc.declare_dram_parameter("x", [B, D], f32, isOutput=False)
    wt_ext = nc.declare_dram_parameter("wt", [D, CS], f32, isOutput=False)
    wlab_ext = nc.declare_dram_parameter("wlab", [B, D], f32, isOutput=False)
    id_ext = nc.declare_dram_parameter("ident", [128, 128], f32, isOutput=False)
    ones_ext = nc.declare_dram_parameter("ones", [128, 128], f32, isOutput=False)
    soff_ext = nc.declare_dram_parameter("soff", [128, 4], i32, isOutput=False)
    out_ext = nc.declare_dram_parameter("out", [B, CS], bf16, isOutput=True)

    x_view = x_ext[:].rearrange("(i p) d -> p i d", p=128)       # [128, 4, 512]
    wl_view = wlab_ext[:].rearrange("(i p) d -> p i d", p=128)   # [128, 4, 512]
    wt_view = wt_ext[:].rearrange("(k p) c -> p k c", p=128)     # [128, 4, CS]
    out_view = out_ext[:].rearrange("(i p) c -> p i c", p=128)   # [128, 4, CS]
    out_flat = out_ext[:].rearrange("b c -> (b c)").unsqueeze(-1)  # [B*CS, 1]

    with tile.TileContext(nc) as tc, ExitStack() as es:
        cpool = es.enter_context(tc.tile_pool(name="consts", bufs=1))
        wpool = es.enter_context(tc.tile_pool(name="wch", bufs=3))
        sqpool = es.enter_context(tc.tile_pool(name="sq", bufs=2))
        wipool = es.enter_context(tc.tile_pool(name="winv", bufs=2))
        outpool = es.enter_context(tc.tile_pool(name="outch", bufs=2))
        ppool = es.enter_context(tc.tile_pool(name="po", bufs=3, space="PSUM"))
        npool = es.enter_context(tc.tile_pool(name="nrm", bufs=1, space="PSUM"))

        identr = cpool.tile([128, 128], f32r, tag="identr")
        nc.sync.dma_start(out=identr[:], in_=id_ext[:].bitcast(f32r))
        onesr = cpool.tile([128, 128], f32r, tag="onesr")
        nc.sync.dma_start(out=onesr[:], in_=ones_ext[:].bitcast(f32r))
        soff_sb = cpool.tile([128, 4], i32, tag="soff_sb")
        nc.sync.dma_start(out=soff_sb[:], in_=soff_ext[:])

        # ---- x path: load, row-normalize (f32), transpose to xnT_k f32r
        x_sb = cpool.tile([128, 4, D], f32, tag="x_sb")
        nc.sync.dma_start(out=x_sb[:], in_=x_view)
        scr = cpool.tile([128, D], bf16, tag="scr")
        ssx = cpool.tile([128, 4], f32, tag="ssx")
        for i in range(4):
            nc.scalar.activation(
                out=scr[:], in_=x_sb[:, i, :], func=Act.Square,
                accum_out=ssx[:, i : i + 1],
            )
        snx = cpool.tile([128, 4], f32, tag="snx")
        nc.scalar.sqrt(snx[:], ssx[:])
        xinv = cpool.tile([128, 4], f32, tag="xinv")
        nc.vector.reciprocal(xinv[:], snx[:])
        xn = cpool.tile([128, 4, D], f32r, tag="xn")
        for i in range(4):
            nc.vector.tensor_scalar_mul(xn[:, i, :], x_sb[:, i, :], xinv[:, i : i + 1])
        xnT = [
            cpool.tile([128, B], f32r, tag=f"xnT{k}", name=f"xnT{k}")
            for k in range(4)
        ]
        for k in range(4):
            pt = npool.tile([128, B], f32r, name="ptx")
            for i in range(4):
                nc.tensor.transpose(
                    pt[:, i * 128 : (i + 1) * 128],
                    xn[:, i, k * 128 : (k + 1) * 128],
                    identr[:],
                )
            nc.vector.tensor_copy(xnT[k][:], pt[:])

        # ---- label path: cos at label from wlab dots, margin phi, val
        wl_sb = cpool.tile([128, 4, D], f32, tag="wl_sb")
        nc.sync.dma_start(out=wl_sb[:], in_=wl_view)
        ssl = cpool.tile([128, 4], f32, tag="ssl")
        for i in range(4):
            nc.scalar.activation(
                out=scr[:], in_=wl_sb[:, i, :], func=Act.Square,
                accum_out=ssl[:, i : i + 1],
            )
        prod = cpool.tile([128, D], f32, tag="prod")
        dot = cpool.tile([128, 4], f32, tag="dot")
        for i in range(4):
            nc.vector.scalar_tensor_tensor(
                out=prod[:], in0=xn[:, i, :].bitcast(f32), scalar=1.0,
                in1=wl_sb[:, i, :], op0=Alu.mult, op1=Alu.mult,
                accum_out=dot[:, i : i + 1],
            )
        snl = cpool.tile([128, 4], f32, tag="snl")
        nc.scalar.sqrt(snl[:], ssl[:])
        slinv = cpool.tile([128, 4], f32, tag="slinv")
        nc.vector.reciprocal(slinv[:], snl[:])
        cosl = cpool.tile([128, 4], f32, tag="cosl")
        nc.vector.tensor_tensor(cosl[:], dot[:], slinv[:], op=Alu.mult)
        # sine = sqrt(max(0, 1 - cos^2)); phi = cos*cos_m - sine*sin_m
        sq = cpool.tile([128, 4], f32, tag="sq4")
        nc.vector.tensor_tensor(sq[:], cosl[:], cosl[:], op=Alu.mult)
        sin2 = cpool.tile([128, 4], f32, tag="sin2")
        nc.vector.tensor_scalar(
            sin2[:], sq[:], -1.0, 1.0, op0=Alu.mult, op1=Alu.add,
        )
        nc.vector.tensor_scalar_max(sin2[:], sin2[:], 0.0)
        sine = cpool.tile([128, 4], f32, tag="sine")
        nc.scalar.sqrt(sine[:], sin2[:])
        t1 = cpool.tile([128, 4], f32, tag="t1")
        nc.vector.tensor_scalar_mul(t1[:], cosl[:], COS_M)
        t2 = cpool.tile([128, 4], f32, tag="t2")
        nc.vector.tensor_scalar_mul(t2[:], sine[:], SIN_M)
        phi = cpool.tile([128, 4], f32, tag="phi")
        nc.vector.tensor_tensor(phi[:], t1[:], t2[:], op=Alu.subtract)
        alt = cpool.tile([128, 4], f32, tag="alt")
        nc.vector.tensor_scalar_sub(alt[:], cosl[:], MM)
        mask = cpool.tile([128, 4], mybir.dt.uint8, tag="mask")
        nc.vector.tensor_scalar(mask[:], cosl[:], TH, None, op0=Alu.is_gt)
        phif = cpool.tile([128, 4], f32, tag="phif")
        nc.vector.tensor_copy(phif[:], alt[:])
        nc.vector.copy_predicated(phif[:], mask[:], phi[:])
        val = cpool.tile([128, 4], bf16, tag="val")
        nc.vector.tensor_scalar_mul(val[:], phif[:], S_SCALE)

        # ---- main loop over class chunks
        c0 = 0
        for cw in CHUNKS:
            wch = wpool.tile([128, 4, 512], f32r, tag="wch")
            nc.sync.dma_start(
                out=wch[:, :, :cw], in_=wt_view[:, :, c0 : c0 + cw].bitcast(f32r)
            )
            # norms: nrm[:, c] = sum_k sum_p wch[p, k, c]^2 (rows replicated)
            nrm = npool.tile([128, 512], f32, name="nrm")
            for k in range(4):
                sqch = sqpool.tile([128, 512], f32r, tag="sqch")
                nc.scalar.activation(
                    out=sqch[:, :cw], in_=wch[:, k, :cw].bitcast(f32),
                    func=Act.Square,
                )
                nc.tensor.matmul(
                    nrm[:, :cw], lhsT=onesr[:], rhs=sqch[:, :cw],
                    start=(k == 0), stop=(k == 3),
                )
            sqt = wipool.tile([128, 512], f32, tag="sqt")
            nc.scalar.activation(
                out=sqt[:, :cw], in_=nrm[:, :cw], func=Act.Sqrt,
                scale=1.0 / (S_SCALE * S_SCALE),
            )
            winvb = wipool.tile([128, 512], f32, tag="winvb")
            nc.vector.reciprocal(winvb[:, :cw], sqt[:, :cw])

            outch = outpool.tile([128, 4, 512], bf16, tag="outch")
            for bp in range(2):
                po = ppool.tile([128, 2, 512], f32, name="po")
                for bbi in range(2):
                    bb = bp * 2 + bbi
                    for k in range(4):
                        nc.tensor.matmul(
                            po[:, bbi, :cw],
                            lhsT=xnT[k][:, bb * 128 : (bb + 1) * 128],
                            rhs=wch[:, k, :cw],
                            start=(k == 0),
                            stop=(k == 3),
                        )
                for bbi in range(2):
                    bb = bp * 2 + bbi
                    nc.vector.tensor_tensor(
                        outch[:, bb, :cw], po[:, bbi, :cw], winvb[:, :cw],
                        op=Alu.mult,
                    )
            nc.sync.dma_start(
                out=out_view[:, :, c0 : c0 + cw], in_=outch[:, :, :cw]
            )
            c0 += cw

        # ---- scatter the 512 margin fixups into out (overwrites s*cos)
        if with_scatter:
            for i in range(4):
                nc.gpsimd.indirect_dma_start(
                    out=out_flat,
                    out_offset=IndirectOffsetOnAxis(
                        ap=soff_sb[:, i : i + 1], axis=0
                    ),
                    in_=val[:, i : i + 1],
                    in_offset=None,
                )

    nc.finalize()
    return nc


def _get_nc():
    if "nc" not in _CACHE:
        _CACHE["nc"] = _build_nc()
    return _CACHE["nc"]


def make_in_maps(x, weight, label):
    x = np.ascontiguousarray(np.asarray(x, dtype=np.float32))
    weight = np.asarray(weight, dtype=np.float32)
    label = np.asarray(label).astype(np.int64)
    wlab = np.ascontiguousarray(weight[label])           # [B, D]
    ident = np.eye(128, dtype=np.float32)
    ones = np.ones((128, 128), dtype=np.float32)
    b_idx = np.arange(B, dtype=np.int64)
    in_maps = []
    for i in range(NCORES):
        a, r = BASE[i], REAL[i]
        wt = np.ones((D, CS), dtype=np.float32)
        wt[:, :r] = weight[a : a + r].T
        loc = label - a
        in_range = (loc >= 0) & (loc < r)
        idx = np.where(in_range, loc, PAD_COL).astype(np.int64)
        soff = (b_idx * CS + idx).astype(np.int32)
        # device layout [128, 4]: column i holds batch rows i*128..i*128+127
        soff_dev = np.ascontiguousarray(soff.reshape(4, 128).T)
        in_maps.append(
            {"x": x, "wt": wt, "wlab": wlab, "ident": ident, "ones": ones,
             "soff": soff_dev}
        )
    return in_maps


def assemble(results):
    shards = [np.asarray(results[i]["out"])[:, : REAL[i]] for i in range(NCORES)]
    return np.concatenate(shards, axis=1).astype(np.float32)  # [B, C]


def kernel(x, weight, label):
    from concourse.bass_utils import run_bass_kernel_spmd

    nc = _get_nc()
    in_maps = make_in_maps(x, weight, label)
    res = run_bass_kernel_spmd(nc, in_maps, list(range(NCORES)))
    return assemble(res.results)
